# revision 8
# baseline (speedup 1.0000x reference)
"""Trainium2 Bass kernel for nn_Attention_39676907885369.

512 independent attention heads (n = a*b = 512). Per head n:
  q_head[128t+r, c] = (x_q[n][t::4, :] @ Wq.T)[c, r] + bq[r]   (the torch
  view(n, c, h) reshape has this 4-strided block structure), same for k, v.
  attn = softmax(q k^T / sqrt(128)); outputs: attn (512,1,512,512) f32 and
  ctxt reshaped to (1, 512, 65536) f32.

Sharding: 64 heads per NeuronCore across 8 cores (head-parallel), weights
replicated. All matmuls on PE in bf16 (f32 PSUM accumulation). Scores are
exponentiated on ACT in both [i,j] and [j,i] layouts (the transposed layout
feeds the ctxt matmul, avoiding on-chip transposes of the attention matrix).
Softmax row sums come for free from a ones-column appended to v in the ctxt
matmul. Normalization is split across DVE and GPSIMD.
"""

import numpy as np
from contextlib import ExitStack

import concourse.bacc as bacc
import concourse.bass as bass
import concourse.mybir as mybir
import concourse.tile as tile
from concourse.bass import ts
from concourse.bass_utils import run_bass_kernel_spmd
from concourse.masks import make_identity

F32 = mybir.dt.float32
BF16 = mybir.dt.bfloat16
N_CORES = 8
N_HEADS = 512
SEQ = 512
DIM = 128
SM_SCALE = float(1.0 / np.sqrt(np.float32(DIM)))
Exp = mybir.ActivationFunctionType.Exp
Identity = mybir.ActivationFunctionType.Identity
VW = DIM + 1  # v block width with the ones column for row sums


def build_program(heads_per_core, ps1_bufs=4, ps2_bufs=2, ps_ctxt_bufs=0,
                  split_convert=1):
    nc = bacc.Bacc(
        "TRN2", target_bir_lowering=False, debug=False, num_devices=N_CORES
    )
    H = heads_per_core

    q_d = nc.dram_tensor("query", [H, SEQ, DIM], F32, kind="ExternalInput").ap()
    k_d = nc.dram_tensor("key", [H, SEQ, DIM], F32, kind="ExternalInput").ap()
    v_d = nc.dram_tensor("value", [H, SEQ, DIM], F32, kind="ExternalInput").ap()
    Wq_d = nc.dram_tensor("Wq", [DIM, DIM], F32, kind="ExternalInput").ap()
    Wk_d = nc.dram_tensor("Wk", [DIM, DIM], F32, kind="ExternalInput").ap()
    Wv_d = nc.dram_tensor("Wv", [DIM, DIM], F32, kind="ExternalInput").ap()
    bq_d = nc.dram_tensor("bq", [1, DIM], F32, kind="ExternalInput").ap()
    bk_d = nc.dram_tensor("bk", [1, DIM], F32, kind="ExternalInput").ap()
    bv_d = nc.dram_tensor("bv", [DIM, 1], F32, kind="ExternalInput").ap()

    attn_d = nc.dram_tensor(
        "attn", [H, SEQ, SEQ], F32, kind="ExternalOutput"
    ).ap()
    out_d = nc.dram_tensor(
        "out", [H, 4, DIM, DIM], F32, kind="ExternalOutput"
    ).ap()

    # [n, c, (t m)] view: rows 4c..4c+3 of head n are one contiguous chunk
    q_v = q_d.rearrange("n (c four) m -> n c (four m)", four=4)
    k_v = k_d.rearrange("n (c four) m -> n c (four m)", four=4)
    v_v = v_d.rearrange("n (c four) m -> n c (four m)", four=4)
    out_v = out_d.rearrange("n t r c -> n r t c")

    with tile.TileContext(nc) as tc, ExitStack() as ctx:
        consts = ctx.enter_context(tc.tile_pool(name="consts", bufs=1))
        ps1 = ctx.enter_context(tc.tile_pool(name="ps1", bufs=ps1_bufs, space="PSUM"))
        ps2 = ctx.enter_context(tc.tile_pool(name="ps2", bufs=ps2_bufs, space="PSUM"))
        psC = (
            ctx.enter_context(tc.tile_pool(name="psC", bufs=ps_ctxt_bufs, space="PSUM"))
            if ps_ctxt_bufs else ps2
        )
        p_xf = ctx.enter_context(tc.tile_pool(name="p_xf", bufs=6))
        p_xb = ctx.enter_context(tc.tile_pool(name="p_xb", bufs=6))
        p_xt = ctx.enter_context(tc.tile_pool(name="p_xt", bufs=4))
        p_qk = ctx.enter_context(tc.tile_pool(name="p_qk", bufs=8))
        p_v = ctx.enter_context(tc.tile_pool(name="p_v", bufs=4))
        p_E = ctx.enter_context(tc.tile_pool(name="p_E", bufs=6))
        p_ET = ctx.enter_context(tc.tile_pool(name="p_ET", bufs=6))
        p_attn = ctx.enter_context(tc.tile_pool(name="p_attn", bufs=12))
        p_ctxt = ctx.enter_context(tc.tile_pool(name="p_ctxt", bufs=4))
        p_rs = ctx.enter_context(tc.tile_pool(name="p_rs", bufs=6))

        # ---- one-time constants ----
        ident = consts.tile([DIM, DIM], BF16)
        make_identity(nc, ident[:])

        # weights: load f32, convert to bf16, transpose on PE -> W?T [m, r]
        WT_sb = consts.tile([DIM, 3 * DIM], BF16)  # [m, (q r | k r | v r)]
        for i, W_d in enumerate((Wq_d, Wk_d, Wv_d)):
            w_f = p_xf.tile([DIM, DIM], F32, tag="wload")
            nc.sync.dma_start(w_f[:], W_d[:])
            w_b = p_xb.tile([DIM, DIM], BF16, tag="wconv")
            nc.vector.tensor_copy(w_b[:], w_f[:])
            wt_ps = ps1.tile([DIM, DIM], BF16, tag="ps1")
            nc.tensor.transpose(wt_ps[:], w_b[:], ident[:])
            nc.vector.tensor_copy(WT_sb[:, ts(i, DIM)], wt_ps[:])
        WqT, WkT, WvT = (WT_sb[:, ts(i, DIM)] for i in range(3))

        # bias broadcast tiles: bqb[c, (t r)] = bq[r]  (f32, [128, 512])
        ones_b = consts.tile([1, DIM], BF16)
        nc.vector.memset(ones_b[:], 1.0)
        bqb = consts.tile([DIM, 4 * DIM], F32)
        bkb = consts.tile([DIM, 4 * DIM], F32)
        for b_d, bb in ((bq_d, bqb), (bk_d, bkb)):
            b_f = p_rs.tile([1, 4 * DIM], F32, tag="bload")
            for t in range(4):
                nc.sync.dma_start(b_f[0:1, ts(t, DIM)], b_d[:])
            b_b = p_rs.tile([1, 4 * DIM], BF16, tag="bconv")
            nc.vector.tensor_copy(b_b[:], b_f[:])
            bb_ps = ps1.tile([DIM, 4 * DIM], F32, tag="ps1")
            nc.tensor.matmul(bb_ps[:], lhsT=ones_b[:], rhs=b_b[:])
            nc.vector.tensor_copy(bb[:], bb_ps[:])
        bv_sb = consts.tile([DIM, 1], F32)
        nc.sync.dma_start(bv_sb[:], bv_d[:])

        # ---- per-head pipeline ----
        for n in range(H):
            # load x slices [c, (t m)] f32, convert to bf16 in one pass
            xf = p_xf.tile([DIM, 12 * DIM], F32, tag="xf")
            nc.sync.dma_start(xf[:, 0 : 4 * DIM], q_v[n])
            nc.sync.dma_start(xf[:, 4 * DIM : 8 * DIM], k_v[n])
            nc.sync.dma_start(xf[:, 8 * DIM : 12 * DIM], v_v[n])
            xb = p_xb.tile([DIM, 12 * DIM], BF16, tag="xb")
            if split_convert:
                nc.gpsimd.tensor_copy(xb[:, 0 : 4 * DIM], xf[:, 0 : 4 * DIM])
                nc.gpsimd.tensor_copy(xb[:, 4 * DIM : 8 * DIM], xf[:, 4 * DIM : 8 * DIM])
                nc.gpsimd.tensor_copy(xb[:, 8 * DIM : 12 * DIM], xf[:, 8 * DIM : 12 * DIM])
            else:
                nc.gpsimd.tensor_copy(xb[:], xf[:])

            # transpose the twelve 128x128 blocks: xt_sb [m, (q|k|v)(t, c)]
            xtqk_ps = ps1.tile([DIM, 8 * DIM], BF16, tag="ps1")
            xtv_ps = ps1.tile([DIM, 4 * DIM], BF16, tag="ps1")
            for t in range(4):
                nc.tensor.transpose(
                    xtqk_ps[:, ts(t, DIM)], xb[:, ts(t, DIM)], ident[:]
                )
                nc.tensor.transpose(
                    xtqk_ps[:, ts(4 + t, DIM)], xb[:, ts(4 + t, DIM)], ident[:]
                )
                nc.tensor.transpose(
                    xtv_ps[:, ts(t, DIM)], xb[:, ts(8 + t, DIM)], ident[:]
                )
            xt_sb = p_xt.tile([DIM, 12 * DIM], BF16, tag="xt")
            nc.vector.tensor_copy(xt_sb[:, 0 : 8 * DIM], xtqk_ps[:])
            nc.vector.tensor_copy(xt_sb[:, 8 * DIM : 12 * DIM], xtv_ps[:])

            # projections
            qT_ps = ps1.tile([DIM, 4 * DIM], F32, tag="ps1")
            kT_ps = ps1.tile([DIM, 4 * DIM], F32, tag="ps1")
            v_ps = ps1.tile([DIM, 4 * DIM], F32, tag="ps1")
            for t in range(4):
                nc.tensor.matmul(
                    qT_ps[:, ts(t, DIM)], lhsT=xt_sb[:, ts(t, DIM)], rhs=WqT
                )
                nc.tensor.matmul(
                    kT_ps[:, ts(t, DIM)], lhsT=xt_sb[:, ts(4 + t, DIM)], rhs=WkT
                )
            nc.tensor.matmul(
                v_ps[:], lhsT=WvT, rhs=xt_sb[:, 8 * DIM : 12 * DIM]
            )

            # bias adds (+ f32->bf16): qT[c, i], kT[c, j]; v[u, (s c)] + ones col
            qT_sb = p_qk.tile([DIM, 4 * DIM], BF16, tag="qkT")
            kT_sb = p_qk.tile([DIM, 4 * DIM], BF16, tag="qkT")
            nc.vector.tensor_add(qT_sb[:], qT_ps[:], bqb[:])
            nc.vector.tensor_add(kT_sb[:], kT_ps[:], bkb[:])
            v_sb = p_v.tile([DIM, 4 * VW], BF16, tag="v")
            v_grid = v_sb[:].rearrange("p (s w) -> p s w", w=VW)
            nc.scalar.activation(
                v_grid[:, :, 0:DIM], v_ps[:], Identity, bias=bv_sb[:]
            )
            nc.gpsimd.memset(v_grid[:, :, DIM:VW], 1.0)

            # scores + exp, [i, j] layout (pairs of i-groups per PSUM tile)
            E_prs = []
            for p in range(2):
                S_ps = ps2.tile([DIM, 8 * DIM], F32, tag="ps2")
                for h in range(2):
                    nc.tensor.matmul(
                        S_ps[:, ts(h, 4 * DIM)],
                        lhsT=qT_sb[:, ts(2 * p + h, DIM)],
                        rhs=kT_sb[:],
                    )
                E_pr = p_E.tile([DIM, 8 * DIM], F32, tag="E")
                nc.scalar.activation(E_pr[:], S_ps[:], Exp, scale=SM_SCALE)
                E_prs.append(E_pr)

            # scores + exp, [j, i] layout (feeds the ctxt matmul)
            ET_prs = []
            for p in range(2):
                ST_ps = ps2.tile([DIM, 8 * DIM], F32, tag="ps2")
                for h in range(2):
                    nc.tensor.matmul(
                        ST_ps[:, ts(h, 4 * DIM)],
                        lhsT=kT_sb[:, ts(2 * p + h, DIM)],
                        rhs=qT_sb[:],
                    )
                ET_pr = p_ET.tile([DIM, 8 * DIM], BF16, tag="ET")
                nc.scalar.activation(ET_pr[:], ST_ps[:], Exp, scale=SM_SCALE)
                ET_prs.append(ET_pr)

            # ctxt[i, (c|rowsum)] = sum_j E[i, j] [v | 1][j, :]
            # blocks of 129 at 256-element strides (never cross a PSUM bank)
            ctxt_ps = psC.tile(
                [DIM, 8 * DIM], F32, tag="psC" if ps_ctxt_bufs else "ps2"
            )
            for t in range(4):
                for s in range(4):
                    off = (s % 2) * 4 * DIM + t * DIM
                    nc.tensor.matmul(
                        ctxt_ps[:, 2 * DIM * t : 2 * DIM * t + VW],
                        lhsT=ET_prs[s // 2][:, off : off + DIM],
                        rhs=v_sb[:, s * VW : (s + 1) * VW],
                        start=(s == 0),
                        stop=(s == 3),
                    )

            # reciprocal of the row sums (strided picks of the ones columns)
            rcp = p_rs.tile([DIM, 4], F32, tag="rcp")
            rs_ap = ctxt_ps[:].rearrange("p (t w) -> p t w", w=2 * DIM)[
                :, :, DIM : DIM + 1
            ]
            nc.vector.reciprocal(rcp[:], rs_ap)

            # attn output rows 128t+i of head n (normalize split DVE/GPSIMD)
            for t in range(4):
                attn_t = p_attn.tile([DIM, 4 * DIM], F32, tag="attn")
                eng = nc.vector if t % 2 == 0 else nc.gpsimd
                eng.tensor_scalar_mul(
                    attn_t[:],
                    E_prs[t // 2][:, ts(t % 2, 4 * DIM)],
                    rcp[:, t : t + 1],
                )
                nc.sync.dma_start(attn_d[n, ts(t, DIM)], attn_t[:])

            # scale ctxt rows by 1/rowsum and store
            ctxt_sb = p_ctxt.tile([DIM, 4 * DIM], F32, tag="ctxt")
            for t in range(4):
                nc.vector.tensor_scalar_mul(
                    ctxt_sb[:, ts(t, DIM)],
                    ctxt_ps[:, 2 * DIM * t : 2 * DIM * t + DIM],
                    rcp[:, t : t + 1],
                )
            nc.sync.dma_start(out_v[n], ctxt_sb[:])

    nc.compile()
    return nc


_PROGRAMS = {}


def get_program(heads_per_core):
    if heads_per_core not in _PROGRAMS:
        _PROGRAMS[heads_per_core] = build_program(heads_per_core)
    return _PROGRAMS[heads_per_core]


def make_in_maps(query, key, value, Wq, bq, Wk, bk, Wv, bv, heads_per_core):
    """Slice full inputs into per-core input maps (head-parallel)."""
    H = heads_per_core
    q = np.ascontiguousarray(np.asarray(query, np.float32).reshape(N_HEADS, SEQ, DIM))
    k = np.ascontiguousarray(np.asarray(key, np.float32).reshape(N_HEADS, SEQ, DIM))
    v = np.ascontiguousarray(np.asarray(value, np.float32).reshape(N_HEADS, SEQ, DIM))
    shared = {
        "Wq": np.asarray(Wq, np.float32),
        "Wk": np.asarray(Wk, np.float32),
        "Wv": np.asarray(Wv, np.float32),
        "bq": np.asarray(bq, np.float32).reshape(1, DIM),
        "bk": np.asarray(bk, np.float32).reshape(1, DIM),
        "bv": np.ascontiguousarray(np.asarray(bv, np.float32).reshape(DIM, 1)),
    }
    in_maps = []
    for c in range(N_CORES):
        sl = slice(c * H, (c + 1) * H)
        in_maps.append(
            {
                "query": np.ascontiguousarray(q[sl]),
                "key": np.ascontiguousarray(k[sl]),
                "value": np.ascontiguousarray(v[sl]),
                **shared,
            }
        )
    return in_maps


def assemble_outputs(results):
    attn = np.concatenate([r["attn"] for r in results], axis=0)
    out = np.concatenate([r["out"] for r in results], axis=0)
    output = out.reshape(N_HEADS, SEQ * DIM)[None]  # (1, 512, 65536)
    return output.reshape(1, SEQ, SEQ * DIM), attn[:, None]


def kernel(query, key, value, Wq, bq, Wk, bk, Wv, bv):
    H = N_HEADS // N_CORES
    nc = get_program(H)
    in_maps = make_in_maps(query, key, value, Wq, bq, Wk, bk, Wv, bv, H)
    res = run_bass_kernel_spmd(nc, in_maps, core_ids=list(range(N_CORES)))
    return assemble_outputs(res.results)
